# revision 2
# baseline (speedup 1.0000x reference)
"""Trainium2 Bass kernel for nn_MultiHeadAttention_61091614818698.

kernel(**inputs) takes FULL unsharded inputs and returns the FULL output
[2,2048,1024]. Tensor-parallel over heads: 2 heads per core on 8 cores.

Pipelined design: per-(rep,batch) projection phases are interleaved into
the previous attention phase's chunk loop (projection PE/DMA work hides
under the exp-bound ACT engine). Scores for both heads are packed into
one [128,1024] PSUM tile (one exp per s-tile); attnV is col-tiled
(tile_position (0,0)/(0,64)) so two M=64 matmuls fill the PE array;
softmax denominators come from DVE/GPSIMD running sums + ones-matmuls;
1/d via DVE reciprocal broadcast through K=1 selection matmuls. bf16
inputs/outputs halve DMA traffic; V is transposed by the DMA XBAR.
"""
import numpy as np
import concourse.bass as bass
import concourse.mybir as mybir
import concourse.tile as tile
from concourse import bacc

F32 = mybir.dt.float32
F32R = mybir.dt.float32r
BF16 = mybir.dt.bfloat16
AF = mybir.ActivationFunctionType
ALU = mybir.AluOpType

POOL_STS = (3, 5, 7, 9, 11)   # odd s-tiles whose esum add runs on GPSIMD


class _Bacc(bacc.Bacc):
    def insert_act_table_loads(self):
        import bass_rust as _br
        from concourse.hw_specs import get_activation_tables
        has_activation = any(
            type(i).__name__ == "InstActivation"
            for b in self.main_func.blocks for i in b.instructions)
        if not has_activation:
            return
        tables = []
        for name, funcs in get_activation_tables(self.m.arch).items():
            if name != "natural_log_exp_and_others":
                funcs = set()
            tables.append((name, funcs))
        _br.insert_act_table_loads(self, tables)


def build_nc(B=2, S=2048, D=1024, HPC=2, use_f32r=True, n_cores=8, repeat=1,
             phases=(1, 2)):
    T = B * S
    DC = D // 128
    QCW = 512
    QC = S // QCW           # 4 chunks per batch
    ST = S // 128           # 16 s-tiles per batch
    GB = S // 128           # token tiles per batch (16)
    MD = F32R if use_f32r else F32

    nc = _Bacc("TRN2", target_bir_lowering=False, debug=False,
               num_devices=n_cores)
    xt = nc.dram_tensor("xt", [D, T], BF16, kind="ExternalInput").ap()
    wq = nc.dram_tensor("wq", [128, D], BF16, kind="ExternalInput").ap()
    wk = nc.dram_tensor("wk", [128, D], BF16, kind="ExternalInput").ap()
    wv = nc.dram_tensor("wv", [128, D], BF16, kind="ExternalInput").ap()
    bq = nc.dram_tensor("bq", [128, 1], F32, kind="ExternalInput").ap()
    bk = nc.dram_tensor("bk", [128, 1], F32, kind="ExternalInput").ap()
    bv = nc.dram_tensor("bv", [128, 1], F32, kind="ExternalInput").ap()
    wo = nc.dram_tensor("wo", [128, D], BF16, kind="ExternalInput").ap()
    ones64 = nc.dram_tensor("ones64", [1, 64], MD, kind="ExternalInput").ap()
    onescol = nc.dram_tensor("onescol", [128, 1], BF16, kind="ExternalInput").ap()
    sel2 = nc.dram_tensor("sel2", [64, 128], MD, kind="ExternalInput").ap()
    po = nc.dram_tensor("po", [T, D], BF16, kind="ExternalOutput").ap()

    with tile.TileContext(nc) as tc:
        with tc.tile_pool(name="singles", bufs=1) as singles, \
             tc.tile_pool(name="kv", bufs=2) as kv, \
             tc.tile_pool(name="xt_pool", bufs=16) as xt_pool, \
             tc.tile_pool(name="vtmp", bufs=2) as vtmp_pool, \
             tc.tile_pool(name="expp", bufs=4) as expp, \
             tc.tile_pool(name="esump", bufs=2) as esump, \
             tc.tile_pool(name="rrp", bufs=2) as rrp, \
             tc.tile_pool(name="o2tp", bufs=2) as o2tp, \
             tc.tile_pool(name="outp", bufs=3) as outp, \
             tc.tile_pool(name="ps_qkv", bufs=1, space="PSUM") as ps_qkv, \
             tc.tile_pool(name="ps_big", bufs=2, space="PSUM") as ps_big, \
             tc.tile_pool(name="ps_o", bufs=1, space="PSUM") as ps_o:

            wq_sb = singles.tile([128, D], BF16, tag="wq")
            wk_sb = singles.tile([128, D], BF16, tag="wk")
            wv_sb = singles.tile([128, D], BF16, tag="wv")
            wo_sb = singles.tile([128, D], BF16, tag="wo")
            bq_sb = singles.tile([128, 1], F32, tag="bq")
            bk_sb = singles.tile([128, 1], F32, tag="bk")
            bv_sb = singles.tile([128, 1], F32, tag="bv")
            o64_sb = singles.tile([1, 64], MD, tag="o64")
            oc_sb = singles.tile([128, 1], BF16, tag="oc")
            sel_sb = singles.tile([64, 128], MD, tag="sel")
            for dst, src in ((wq_sb, wq), (wk_sb, wk), (wv_sb, wv),
                             (wo_sb, wo), (bq_sb, bq), (bk_sb, bk),
                             (bv_sb, bv), (o64_sb, ones64), (oc_sb, onescol),
                             (sel_sb, sel2)):
                nc.sync.dma_start(out=dst[:], in_=src[:])

            def new_slot():
                qt = kv.tile([128, S], MD, tag="qt2")
                kt = kv.tile([128, S], MD, tag="kt2")
                vsb = kv.tile([128, GB * 128], BF16, tag="vs")
                return {"q": qt, "k": kt,
                        "v": vsb[:].rearrange("p (g n) -> p g n", n=128)}

            def emit_proj_tch(slot, b, tch):
                """Project one 512-token group of batch b into slot."""
                tsl = bass.ds(b * S + tch * 512, 512)
                lsl = bass.ts(tch, 512)    # local (within-batch) columns
                xx = []
                for dc in range(DC):
                    xtile = xt_pool.tile([128, 512], BF16, tag="xt")
                    nc.sync.dma_start(
                        out=xtile[:], in_=xt[dc * 128:(dc + 1) * 128, tsl])
                    xx.append(xtile)
                pq = ps_qkv.tile([128, 512], F32, tag="pq")
                pk = ps_qkv.tile([128, 512], F32, tag="pk")
                pv = ps_qkv.tile([128, 512], F32, tag="pv")
                for dc in range(DC):
                    dsl = bass.ts(dc, 128)
                    st_, sp_ = dc == 0, dc == DC - 1
                    nc.tensor.matmul(pq[:], wq_sb[:, dsl], xx[dc][:],
                                     start=st_, stop=sp_)
                    nc.tensor.matmul(pk[:], wk_sb[:, dsl], xx[dc][:],
                                     start=st_, stop=sp_)
                    nc.tensor.matmul(pv[:], wv_sb[:, dsl], xx[dc][:],
                                     start=st_, stop=sp_)
                nc.scalar.activation(slot["q"][:, lsl], pq[:],
                                     AF.Identity, bias=bq_sb[:])
                nc.scalar.activation(slot["k"][:, lsl], pk[:],
                                     AF.Identity, bias=bk_sb[:])
                vt = vtmp_pool.tile([128, 512], BF16, tag="vt")
                nc.scalar.activation(vt[:], pv[:], AF.Identity, bias=bv_sb[:])
                for i in range(4):
                    nc.sync.dma_start(out=slot["v"][:, tch * 4 + i, :],
                                      in_=vt[:, bass.ts(i, 128)],
                                      transpose=True)

            if 1 not in phases or 2 not in phases:
                raise ValueError("v3 requires both phases")

            # ---- global pipelined schedule ----
            phase_list = [(rep, b) for rep in range(repeat) for b in range(B)]
            nphase = len(phase_list)
            chunks = [(pi, qc) for pi in range(nphase) for qc in range(QC)]
            nchunk = len(chunks)

            slots = {0: new_slot()}
            for tch in range(4):
                emit_proj_tch(slots[0], phase_list[0][1], tch)

            pend = {}

            def queue_scores(ci, st):
                pi, qc = chunks[ci]
                slot = slots[pi]
                ps = ps_big.tile([128, 2 * QCW], F32, tag="ps")
                for h in range(HPC):
                    hp = h * 64
                    nc.tensor.matmul(
                        ps[:, h * QCW:(h + 1) * QCW],
                        slot["k"][hp:hp + 64, bass.ds(st * 128, 128)],
                        slot["q"][hp:hp + 64, bass.ds(qc * QCW, QCW)],
                        start=True, stop=True,
                        tile_position=(hp, 0),
                        skip_group_check=True)
                pend[(ci, st)] = ps

            queue_scores(0, 0)
            queue_scores(0, 1)

            for ci in range(nchunk):
                pi, qc = chunks[ci]
                rep, b = phase_list[pi]
                slot = slots[pi]
                # allocate next phase's slot right before its proj starts
                if qc == 0 and pi + 1 < nphase and (pi + 1) not in slots:
                    slots[pi + 1] = new_slot()

                o2t = o2tp.tile([128, QCW], BF16, tag="o2t")
                oacc = ps_o.tile([128, QCW], F32, tag="oacc")
                esA = esump.tile([128, 2 * QCW], BF16, tag="esA")
                esB = esump.tile([128, 2 * QCW], BF16, tag="esB")
                etiles = {}

                for st in range(ST):
                    ps = pend.pop((ci, st))
                    e = expp.tile([128, 2 * QCW], BF16, tag="e")
                    etiles[st] = e
                    nc.scalar.activation(e[:], ps[:], AF.Exp, scale=0.125)
                    for h in range(HPC):
                        nc.tensor.matmul(
                            oacc[h * 64:(h + 1) * 64, :],
                            slot["v"][:, st, h * 64:(h + 1) * 64],
                            e[:, h * QCW:(h + 1) * QCW],
                            start=(st == 0), stop=(st == ST - 1),
                            tile_position=(0, h * 64),
                            skip_group_check=True)
                    # esum chains: DVE on esA (sts 0,2,4..14,13,15),
                    # GPSIMD on esB (sts 1,3,5,7,9,11)
                    if st == 2:
                        nc.vector.tensor_tensor(out=esA[:], in0=etiles[0][:],
                                                in1=e[:], op=ALU.add)
                        etiles.pop(0)
                    elif st == 3:
                        nc.gpsimd.tensor_tensor(out=esB[:], in0=etiles[1][:],
                                                in1=e[:], op=ALU.add)
                        etiles.pop(1)
                    elif st in POOL_STS:
                        nc.gpsimd.tensor_tensor(out=esB[:], in0=esB[:],
                                                in1=e[:], op=ALU.add)
                    elif st > 3:
                        nc.vector.tensor_tensor(out=esA[:], in0=esA[:],
                                                in1=e[:], op=ALU.add)

                    # pipeline: keep 2 score tiles in flight, across chunks
                    nst = st + 2
                    if nst < ST:
                        queue_scores(ci, nst)
                    elif ci + 1 < nchunk:
                        queue_scores(ci + 1, nst - ST)

                    # interleave next phase's projections (one 512-token
                    # group per chunk, mid-chunk)
                    if st == 8 and pi + 1 < nphase:
                        emit_proj_tch(slots[pi + 1], phase_list[pi + 1][1], qc)

                # ---- chunk tail ----
                rr0t = rrp.tile([1, QCW], MD, tag="rr0")
                rr1t = rrp.tile([64, QCW], MD, tag="rr1")
                rrs = [rr0t[:], rr1t[32:33, :]]
                for h in range(HPC):
                    pdt = ps_big.tile([128, 2 * QCW], F32, tag="ps")
                    for j, es in enumerate((esA, esB)):
                        nc.tensor.matmul(
                            pdt[0:1, 0:QCW], oc_sb[:],
                            es[:, h * QCW:(h + 1) * QCW],
                            start=(j == 0), stop=(j == 1),
                            skip_group_check=True)
                    with nc.allow_low_precision(reason="1/d"):
                        nc.vector.reciprocal(rrs[h], pdt[0:1, 0:QCW])
                rbt = ps_big.tile([128, 2 * QCW], F32, tag="ps")
                rb = rbt[:, 0:QCW]
                for h in range(HPC):
                    nc.tensor.matmul(
                        rb[:], sel_sb[h * 32:h * 32 + 1, :], rrs[h],
                        start=(h == 0), stop=(h == HPC - 1),
                        skip_group_check=True)
                rbsb = rrp.tile([128, QCW], F32, tag="rbsb")
                nc.vector.tensor_copy(rbsb[:], rb[:])
                nc.vector.tensor_tensor(out=o2t[:], in0=oacc[:], in1=rbsb[:],
                                        op=ALU.mult)
                for i in range(QCW // 128):
                    gt = b * GB + qc * (QCW // 128) + i
                    ppt = ps_big.tile([128, 2 * QCW], F32, tag="ps")
                    for ec in range(2):
                        nc.tensor.matmul(
                            ppt[:, ec * 512:(ec + 1) * 512],
                            o2t[:, bass.ts(i, 128)],
                            wo_sb[:, bass.ts(ec, 512)],
                            start=True, stop=True,
                            skip_group_check=True)
                    ot = outp.tile([128, D], BF16, tag="ot")
                    nc.vector.tensor_copy(ot[:], ppt[:])
                    nc.sync.dma_start(
                        out=po[gt * 128:(gt + 1) * 128, 0:D],
                        in_=ot[:])
                if pi in slots and qc == QC - 1:
                    slots.pop(pi)
    nc.compile()
    return nc


def host_inputs(x, Wq, bqv, Wk, bkv, Wv, bvv, Wo, n_cores=8, hpc=2):
    import ml_dtypes
    BF = ml_dtypes.bfloat16
    B, S, D = x.shape
    T = B * S
    xt = np.ascontiguousarray(x.reshape(T, D).T).astype(BF)
    ones64 = np.ones((1, 64), dtype=np.float32)
    wot = np.ascontiguousarray(Wo.T).astype(np.float32)

    def wpack(W, c):
        W2 = np.concatenate([W[hpc * c + j] for j in range(hpc)], axis=1)
        return np.ascontiguousarray(
            W2.reshape(D // 128, 128, 128).transpose(1, 0, 2).reshape(128, D)
        ).astype(BF)

    def bpack(bb, c):
        return np.concatenate([bb[hpc * c + j] for j in range(hpc)]
                              ).reshape(128, 1).astype(np.float32)

    sel2np = np.zeros((64, 128), dtype=np.float32)
    sel2np[0, 0:64] = 1.0
    sel2np[32, 64:128] = 1.0
    maps = []
    for c in range(n_cores):
        maps.append({
            "xt": xt,
            "wq": wpack(Wq, c), "wk": wpack(Wk, c), "wv": wpack(Wv, c),
            "bq": bpack(bqv, c), "bk": bpack(bkv, c), "bv": bpack(bvv, c),
            "wo": np.ascontiguousarray(wot[c * 128:(c + 1) * 128, :]).astype(BF),
            "ones64": ones64,
            "onescol": np.ones((128, 1), dtype=BF),
            "sel2": sel2np,
        })
    return maps


class Runner:
    """Compile once, run many times through the PJRT/axon path."""

    def __init__(self, nc, n_cores=8):
        import jax
        import numpy as _np
        from jax.sharding import Mesh, PartitionSpec
        from jax.experimental.shard_map import shard_map
        from concourse import bass2jax, mybir as _mybir
        bass2jax.install_neuronx_cc_hook()
        self.jax = jax
        self.n_cores = n_cores
        partition_name = (nc.partition_id_tensor.name
                          if nc.partition_id_tensor else None)
        self.partition_name = partition_name
        in_names, out_names, out_avals, zero_outs = [], [], [], []
        for alloc in nc.m.functions[0].allocations:
            if not isinstance(alloc, _mybir.MemoryLocationSet):
                continue
            name = alloc.memorylocations[0].name
            if alloc.kind == "ExternalInput":
                if name != partition_name:
                    in_names.append(name)
            elif alloc.kind == "ExternalOutput":
                out_names.append(name)
                shape = tuple(alloc.tensor_shape)
                dtype = _mybir.dt.np(alloc.dtype)
                out_avals.append(jax.core.ShapedArray(shape, dtype))
                zero_outs.append((shape, dtype))
        self.in_names, self.out_names = list(in_names), list(out_names)
        self.out_avals, self.zero_shapes = out_avals, zero_outs
        n_params, n_outs = len(in_names), len(out_names)
        self.n_params = n_params
        all_names = in_names + out_names
        if partition_name is not None:
            all_names = all_names + [partition_name]

        def _body(*args):
            operands = list(args)
            if partition_name is not None:
                operands.append(bass2jax.partition_id_tensor())
            outs = bass2jax._bass_exec_p.bind(
                *operands,
                out_avals=tuple(out_avals),
                in_names=tuple(all_names),
                out_names=tuple(out_names),
                lowering_input_output_aliases=(),
                sim_require_finite=True,
                sim_require_nnan=True,
                nc=nc,
            )
            return tuple(outs)

        devices = jax.devices()[:n_cores]
        self.mesh = Mesh(_np.asarray(devices), ("core",))
        self.pspec = PartitionSpec("core")
        in_specs = (self.pspec,) * (n_params + n_outs)
        out_specs = (self.pspec,) * n_outs
        self.donate = tuple(range(n_params, n_params + n_outs))
        self.fn = jax.jit(
            shard_map(_body, mesh=self.mesh, in_specs=in_specs,
                      out_specs=out_specs, check_rep=False),
            donate_argnums=self.donate, keep_unused=True)

    def stage_inputs(self, in_maps):
        import numpy as _np
        from jax.sharding import NamedSharding
        sh = NamedSharding(self.mesh, self.pspec)
        staged = []
        for name in self.in_names:
            g = _np.concatenate([_np.asarray(m[name]) for m in in_maps],
                                axis=0)
            staged.append(self.jax.device_put(g, sh))
        return staged

    def make_zeros(self):
        import numpy as _np
        from jax.sharding import NamedSharding
        sh = NamedSharding(self.mesh, self.pspec)
        return [self.jax.device_put(
                    _np.zeros((self.n_cores * s[0], *s[1:]), d), sh)
                for (s, d) in self.zero_shapes]

    def run(self, staged_in, zeros):
        return self.fn(*staged_in, *zeros)

    def results(self, outs):
        import numpy as _np
        res = []
        for c in range(self.n_cores):
            d = {}
            for i, name in enumerate(self.out_names):
                a = self.out_avals[i]
                d[name] = _np.asarray(outs[i]).reshape(
                    self.n_cores, *a.shape)[c]
            res.append(d)
        return res


_STATE = {}


def _get_runner():
    if "runner" not in _STATE:
        nc = build_nc(B=2, S=2048, D=1024, HPC=2, use_f32r=True, n_cores=8,
                      repeat=1, phases=(1, 2))
        _STATE["runner"] = Runner(nc, n_cores=8)
    return _STATE["runner"]


def kernel(x, Wq, bq, Wk, bk, Wv, bv, Wo, bo):
    import numpy as _np
    x = _np.asarray(x, dtype=_np.float32)
    Wq = _np.asarray(Wq, dtype=_np.float32)
    bq_ = _np.asarray(bq, dtype=_np.float32)
    Wk = _np.asarray(Wk, dtype=_np.float32)
    bk_ = _np.asarray(bk, dtype=_np.float32)
    Wv = _np.asarray(Wv, dtype=_np.float32)
    bv_ = _np.asarray(bv, dtype=_np.float32)
    Wo = _np.asarray(Wo, dtype=_np.float32)
    bo_ = _np.asarray(bo, dtype=_np.float32)
    B, S, D = x.shape
    r = _get_runner()
    maps = host_inputs(x, Wq, bq_, Wk, bk_, Wv, bv_, Wo)
    staged = r.stage_inputs(maps)
    outs = r.run(staged, r.make_zeros())
    res = r.results(outs)
    acc = _np.zeros((B * S, D), dtype=_np.float32)
    for c in range(8):
        acc += res[c]["po"].astype(_np.float32)
    return (acc.reshape(B, S, D) + bo_).astype(_np.float32)



# revision 3
# speedup vs baseline: 4.5926x; 4.5926x over previous
"""Trainium2 Bass kernel for nn_MultiHeadAttention_61091614818698.

kernel(**inputs) takes FULL unsharded inputs and returns the FULL output
[2,2048,1024]. Tensor-parallel over heads: 2 heads per core on 8 cores.

Pipelined design: per-(rep,batch) projection phases are interleaved into
the previous attention phase's chunk loop (projection PE/DMA work hides
under the exp-bound ACT engine). Scores for both heads are packed into
one [128,1024] PSUM tile (one exp per s-tile); attnV is col-tiled
(tile_position (0,0)/(0,64)) so two M=64 matmuls fill the PE array;
softmax denominators come from DVE/GPSIMD running sums + ones-matmuls;
1/d via DVE reciprocal broadcast through K=1 selection matmuls. bf16
inputs/outputs halve DMA traffic; V is transposed by the DMA XBAR.
"""
import numpy as np
import concourse.bass as bass
import concourse.mybir as mybir
import concourse.tile as tile
from concourse import bacc

F32 = mybir.dt.float32
F32R = mybir.dt.float32r
BF16 = mybir.dt.bfloat16
AF = mybir.ActivationFunctionType
ALU = mybir.AluOpType

POOL_STS = (3, 5, 7, 9, 11)   # odd s-tiles whose esum add runs on GPSIMD


class _Bacc(bacc.Bacc):
    def insert_act_table_loads(self):
        import bass_rust as _br
        from concourse.hw_specs import get_activation_tables
        has_activation = any(
            type(i).__name__ == "InstActivation"
            for b in self.main_func.blocks for i in b.instructions)
        if not has_activation:
            return
        tables = []
        for name, funcs in get_activation_tables(self.m.arch).items():
            if name != "natural_log_exp_and_others":
                funcs = set()
            tables.append((name, funcs))
        _br.insert_act_table_loads(self, tables)


def build_nc(B=2, S=2048, D=1024, HPC=2, use_f32r=True, n_cores=8, repeat=1,
             phases=(1, 2)):
    T = B * S
    DC = D // 128
    QCW = 512
    QC = S // QCW           # 4 chunks per batch
    ST = S // 128           # 16 s-tiles per batch
    GB = S // 128           # token tiles per batch (16)
    MD = F32R if use_f32r else F32

    nc = _Bacc("TRN2", target_bir_lowering=False, debug=False,
               num_devices=n_cores)
    xt = nc.dram_tensor("xt", [D, T], BF16, kind="ExternalInput").ap()
    wq = nc.dram_tensor("wq", [128, D], BF16, kind="ExternalInput").ap()
    wk = nc.dram_tensor("wk", [128, D], BF16, kind="ExternalInput").ap()
    wv = nc.dram_tensor("wv", [128, D], BF16, kind="ExternalInput").ap()
    bq = nc.dram_tensor("bq", [128, 1], F32, kind="ExternalInput").ap()
    bk = nc.dram_tensor("bk", [128, 1], F32, kind="ExternalInput").ap()
    bv = nc.dram_tensor("bv", [128, 1], F32, kind="ExternalInput").ap()
    wo = nc.dram_tensor("wo", [128, D], BF16, kind="ExternalInput").ap()
    ones64 = nc.dram_tensor("ones64", [1, 64], MD, kind="ExternalInput").ap()
    onescol = nc.dram_tensor("onescol", [128, 1], BF16, kind="ExternalInput").ap()
    sel2 = nc.dram_tensor("sel2", [64, 128], MD, kind="ExternalInput").ap()
    po = nc.dram_tensor("po", [T, D], BF16, kind="ExternalOutput").ap()

    with tile.TileContext(nc) as tc:
        with tc.tile_pool(name="singles", bufs=1) as singles, \
             tc.tile_pool(name="kv", bufs=2) as kv, \
             tc.tile_pool(name="xt_pool", bufs=16) as xt_pool, \
             tc.tile_pool(name="vtmp", bufs=2) as vtmp_pool, \
             tc.tile_pool(name="expp", bufs=4) as expp, \
             tc.tile_pool(name="esump", bufs=2) as esump, \
             tc.tile_pool(name="rrp", bufs=2) as rrp, \
             tc.tile_pool(name="o2tp", bufs=2) as o2tp, \
             tc.tile_pool(name="outp", bufs=3) as outp, \
             tc.tile_pool(name="ps_qkv", bufs=1, space="PSUM") as ps_qkv, \
             tc.tile_pool(name="ps_big", bufs=2, space="PSUM") as ps_big, \
             tc.tile_pool(name="ps_o", bufs=1, space="PSUM") as ps_o:

            wq_sb = singles.tile([128, D], BF16, tag="wq")
            wk_sb = singles.tile([128, D], BF16, tag="wk")
            wv_sb = singles.tile([128, D], BF16, tag="wv")
            wo_sb = singles.tile([128, D], BF16, tag="wo")
            bq_sb = singles.tile([128, 1], F32, tag="bq")
            bk_sb = singles.tile([128, 1], F32, tag="bk")
            bv_sb = singles.tile([128, 1], F32, tag="bv")
            o64_sb = singles.tile([1, 64], MD, tag="o64")
            oc_sb = singles.tile([128, 1], BF16, tag="oc")
            sel_sb = singles.tile([64, 128], MD, tag="sel")
            for dst, src in ((wq_sb, wq), (wk_sb, wk), (wv_sb, wv),
                             (wo_sb, wo), (bq_sb, bq), (bk_sb, bk),
                             (bv_sb, bv), (o64_sb, ones64), (oc_sb, onescol),
                             (sel_sb, sel2)):
                nc.sync.dma_start(out=dst[:], in_=src[:])

            def new_slot():
                qt = kv.tile([128, S], MD, tag="qt2")
                kt = kv.tile([128, S], MD, tag="kt2")
                vsb = kv.tile([128, GB * 128], BF16, tag="vs")
                return {"q": qt, "k": kt,
                        "v": vsb[:].rearrange("p (g n) -> p g n", n=128)}

            def emit_proj_tch(slot, b, tch):
                """Project one 512-token group of batch b into slot."""
                tsl = bass.ds(b * S + tch * 512, 512)
                lsl = bass.ts(tch, 512)    # local (within-batch) columns
                xx = []
                for dc in range(DC):
                    xtile = xt_pool.tile([128, 512], BF16, tag="xt")
                    nc.sync.dma_start(
                        out=xtile[:], in_=xt[dc * 128:(dc + 1) * 128, tsl])
                    xx.append(xtile)
                pq = ps_qkv.tile([128, 512], F32, tag="pq")
                pk = ps_qkv.tile([128, 512], F32, tag="pk")
                pv = ps_qkv.tile([128, 512], F32, tag="pv")
                for dc in range(DC):
                    dsl = bass.ts(dc, 128)
                    st_, sp_ = dc == 0, dc == DC - 1
                    nc.tensor.matmul(pq[:], wq_sb[:, dsl], xx[dc][:],
                                     start=st_, stop=sp_)
                    nc.tensor.matmul(pk[:], wk_sb[:, dsl], xx[dc][:],
                                     start=st_, stop=sp_)
                    nc.tensor.matmul(pv[:], wv_sb[:, dsl], xx[dc][:],
                                     start=st_, stop=sp_)
                nc.scalar.activation(slot["q"][:, lsl], pq[:],
                                     AF.Identity, bias=bq_sb[:])
                nc.scalar.activation(slot["k"][:, lsl], pk[:],
                                     AF.Identity, bias=bk_sb[:])
                vt = vtmp_pool.tile([128, 512], BF16, tag="vt")
                nc.scalar.activation(vt[:], pv[:], AF.Identity, bias=bv_sb[:])
                for i in range(4):
                    nc.sync.dma_start(out=slot["v"][:, tch * 4 + i, :],
                                      in_=vt[:, bass.ts(i, 128)],
                                      transpose=True)

            if 1 not in phases or 2 not in phases:
                raise ValueError("v3 requires both phases")

            # ---- global pipelined schedule ----
            phase_list = [(rep, b) for rep in range(repeat) for b in range(B)]
            nphase = len(phase_list)
            chunks = [(pi, qc) for pi in range(nphase) for qc in range(QC)]
            nchunk = len(chunks)

            slots = {0: new_slot()}
            for tch in range(4):
                emit_proj_tch(slots[0], phase_list[0][1], tch)

            pend = {}

            def queue_scores(ci, st):
                pi, qc = chunks[ci]
                slot = slots[pi]
                ps = ps_big.tile([128, 2 * QCW], F32, tag="ps")
                for h in range(HPC):
                    hp = h * 64
                    nc.tensor.matmul(
                        ps[:, h * QCW:(h + 1) * QCW],
                        slot["k"][hp:hp + 64, bass.ds(st * 128, 128)],
                        slot["q"][hp:hp + 64, bass.ds(qc * QCW, QCW)],
                        start=True, stop=True,
                        tile_position=(hp, 0),
                        skip_group_check=True)
                pend[(ci, st)] = ps

            for _pre in range(4):
                queue_scores(0, _pre)

            for ci in range(nchunk):
                pi, qc = chunks[ci]
                rep, b = phase_list[pi]
                slot = slots[pi]
                # allocate next phase's slot right before its proj starts
                if qc == 0 and pi + 1 < nphase and (pi + 1) not in slots:
                    slots[pi + 1] = new_slot()

                o2t = o2tp.tile([128, QCW], BF16, tag="o2t")
                oacc = ps_o.tile([128, QCW], F32, tag="oacc")
                esA = esump.tile([128, 2 * QCW], BF16, tag="esA")
                esB = esump.tile([128, 2 * QCW], BF16, tag="esB")
                etiles = {}

                for st in range(ST):
                    ps = pend.pop((ci, st), None)
                    if ps is None:
                        queue_scores(ci, st)
                        ps = pend.pop((ci, st))
                    e = expp.tile([128, 2 * QCW], BF16, tag="e")
                    etiles[st] = e
                    nc.scalar.activation(e[:], ps[:], AF.Exp, scale=0.125)
                    for h in range(HPC):
                        nc.tensor.matmul(
                            oacc[h * 64:(h + 1) * 64, :],
                            slot["v"][:, st, h * 64:(h + 1) * 64],
                            e[:, h * QCW:(h + 1) * QCW],
                            start=(st == 0), stop=(st == ST - 1),
                            tile_position=(0, h * 64),
                            skip_group_check=True)
                    # esum chains: DVE on esA (sts 0,2,4..14,13,15),
                    # GPSIMD on esB (sts 1,3,5,7,9,11)
                    if st == 2:
                        nc.vector.tensor_tensor(out=esA[:], in0=etiles[0][:],
                                                in1=e[:], op=ALU.add)
                        etiles.pop(0)
                    elif st == 3:
                        nc.gpsimd.tensor_tensor(out=esB[:], in0=etiles[1][:],
                                                in1=e[:], op=ALU.add)
                        etiles.pop(1)
                    elif st in POOL_STS:
                        nc.gpsimd.tensor_tensor(out=esB[:], in0=esB[:],
                                                in1=e[:], op=ALU.add)
                    elif st > 3:
                        nc.vector.tensor_tensor(out=esA[:], in0=esA[:],
                                                in1=e[:], op=ALU.add)

                    # pipeline: keep 2 score tiles in flight, across chunks
                    nst = st + 4
                    if nst < ST:
                        if (ci, nst) not in pend:
                            queue_scores(ci, nst)
                    elif ci + 1 < nchunk and (ci + 1, nst - ST) not in pend:
                        queue_scores(ci + 1, nst - ST)

                    # interleave next phase's projections (one 512-token
                    # group per chunk, mid-chunk)
                    if st == 4 and pi + 1 < nphase:
                        emit_proj_tch(slots[pi + 1], phase_list[pi + 1][1], qc)

                # ---- chunk tail ----
                rr0t = rrp.tile([1, QCW], MD, tag="rr0")
                rr1t = rrp.tile([64, QCW], MD, tag="rr1")
                rrs = [rr0t[:], rr1t[32:33, :]]
                for h in range(HPC):
                    pdt = ps_big.tile([128, 2 * QCW], F32, tag="ps")
                    for j, es in enumerate((esA, esB)):
                        nc.tensor.matmul(
                            pdt[0:1, 0:QCW], oc_sb[:],
                            es[:, h * QCW:(h + 1) * QCW],
                            start=(j == 0), stop=(j == 1),
                            skip_group_check=True)
                    with nc.allow_low_precision(reason="1/d"):
                        nc.vector.reciprocal(rrs[h], pdt[0:1, 0:QCW])
                rbt = ps_big.tile([128, 2 * QCW], F32, tag="ps")
                rb = rbt[:, 0:QCW]
                for h in range(HPC):
                    nc.tensor.matmul(
                        rb[:], sel_sb[h * 32:h * 32 + 1, :], rrs[h],
                        start=(h == 0), stop=(h == HPC - 1),
                        skip_group_check=True)
                rbsb = rrp.tile([128, QCW], F32, tag="rbsb")
                nc.vector.tensor_copy(rbsb[:], rb[:])
                nc.vector.tensor_tensor(out=o2t[:], in0=oacc[:], in1=rbsb[:],
                                        op=ALU.mult)
                for i in range(QCW // 128):
                    gt = b * GB + qc * (QCW // 128) + i
                    ppt = ps_big.tile([128, 2 * QCW], F32, tag="ps")
                    for ec in range(2):
                        nc.tensor.matmul(
                            ppt[:, ec * 512:(ec + 1) * 512],
                            o2t[:, bass.ts(i, 128)],
                            wo_sb[:, bass.ts(ec, 512)],
                            start=True, stop=True,
                            skip_group_check=True)
                    ot = outp.tile([128, D], BF16, tag="ot")
                    nc.vector.tensor_copy(ot[:], ppt[:])
                    nc.sync.dma_start(
                        out=po[gt * 128:(gt + 1) * 128, 0:D],
                        in_=ot[:])
                if pi in slots and qc == QC - 1:
                    slots.pop(pi)
    nc.compile()
    return nc


def host_inputs(x, Wq, bqv, Wk, bkv, Wv, bvv, Wo, n_cores=8, hpc=2):
    import ml_dtypes
    BF = ml_dtypes.bfloat16
    B, S, D = x.shape
    T = B * S
    xt = np.ascontiguousarray(x.reshape(T, D).T).astype(BF)
    ones64 = np.ones((1, 64), dtype=np.float32)
    wot = np.ascontiguousarray(Wo.T).astype(np.float32)

    def wpack(W, c):
        W2 = np.concatenate([W[hpc * c + j] for j in range(hpc)], axis=1)
        return np.ascontiguousarray(
            W2.reshape(D // 128, 128, 128).transpose(1, 0, 2).reshape(128, D)
        ).astype(BF)

    def bpack(bb, c):
        return np.concatenate([bb[hpc * c + j] for j in range(hpc)]
                              ).reshape(128, 1).astype(np.float32)

    sel2np = np.zeros((64, 128), dtype=np.float32)
    sel2np[0, 0:64] = 1.0
    sel2np[32, 64:128] = 1.0
    maps = []
    for c in range(n_cores):
        maps.append({
            "xt": xt,
            "wq": wpack(Wq, c), "wk": wpack(Wk, c), "wv": wpack(Wv, c),
            "bq": bpack(bqv, c), "bk": bpack(bkv, c), "bv": bpack(bvv, c),
            "wo": np.ascontiguousarray(wot[c * 128:(c + 1) * 128, :]).astype(BF),
            "ones64": ones64,
            "onescol": np.ones((128, 1), dtype=BF),
            "sel2": sel2np,
        })
    return maps


class Runner:
    """Compile once, run many times through the PJRT/axon path."""

    def __init__(self, nc, n_cores=8):
        import jax
        import numpy as _np
        from jax.sharding import Mesh, PartitionSpec
        from jax.experimental.shard_map import shard_map
        from concourse import bass2jax, mybir as _mybir
        bass2jax.install_neuronx_cc_hook()
        self.jax = jax
        self.n_cores = n_cores
        partition_name = (nc.partition_id_tensor.name
                          if nc.partition_id_tensor else None)
        self.partition_name = partition_name
        in_names, out_names, out_avals, zero_outs = [], [], [], []
        for alloc in nc.m.functions[0].allocations:
            if not isinstance(alloc, _mybir.MemoryLocationSet):
                continue
            name = alloc.memorylocations[0].name
            if alloc.kind == "ExternalInput":
                if name != partition_name:
                    in_names.append(name)
            elif alloc.kind == "ExternalOutput":
                out_names.append(name)
                shape = tuple(alloc.tensor_shape)
                dtype = _mybir.dt.np(alloc.dtype)
                out_avals.append(jax.core.ShapedArray(shape, dtype))
                zero_outs.append((shape, dtype))
        self.in_names, self.out_names = list(in_names), list(out_names)
        self.out_avals, self.zero_shapes = out_avals, zero_outs
        n_params, n_outs = len(in_names), len(out_names)
        self.n_params = n_params
        all_names = in_names + out_names
        if partition_name is not None:
            all_names = all_names + [partition_name]

        def _body(*args):
            operands = list(args)
            if partition_name is not None:
                operands.append(bass2jax.partition_id_tensor())
            outs = bass2jax._bass_exec_p.bind(
                *operands,
                out_avals=tuple(out_avals),
                in_names=tuple(all_names),
                out_names=tuple(out_names),
                lowering_input_output_aliases=(),
                sim_require_finite=True,
                sim_require_nnan=True,
                nc=nc,
            )
            return tuple(outs)

        devices = jax.devices()[:n_cores]
        self.mesh = Mesh(_np.asarray(devices), ("core",))
        self.pspec = PartitionSpec("core")
        in_specs = (self.pspec,) * (n_params + n_outs)
        out_specs = (self.pspec,) * n_outs
        self.donate = tuple(range(n_params, n_params + n_outs))
        self.fn = jax.jit(
            shard_map(_body, mesh=self.mesh, in_specs=in_specs,
                      out_specs=out_specs, check_rep=False),
            donate_argnums=self.donate, keep_unused=True)

    def stage_inputs(self, in_maps):
        import numpy as _np
        from jax.sharding import NamedSharding
        sh = NamedSharding(self.mesh, self.pspec)
        staged = []
        for name in self.in_names:
            g = _np.concatenate([_np.asarray(m[name]) for m in in_maps],
                                axis=0)
            staged.append(self.jax.device_put(g, sh))
        return staged

    def make_zeros(self):
        import numpy as _np
        from jax.sharding import NamedSharding
        sh = NamedSharding(self.mesh, self.pspec)
        return [self.jax.device_put(
                    _np.zeros((self.n_cores * s[0], *s[1:]), d), sh)
                for (s, d) in self.zero_shapes]

    def run(self, staged_in, zeros):
        return self.fn(*staged_in, *zeros)

    def results(self, outs):
        import numpy as _np
        res = []
        for c in range(self.n_cores):
            d = {}
            for i, name in enumerate(self.out_names):
                a = self.out_avals[i]
                d[name] = _np.asarray(outs[i]).reshape(
                    self.n_cores, *a.shape)[c]
            res.append(d)
        return res


_STATE = {}


def _get_runner():
    if "runner" not in _STATE:
        nc = build_nc(B=2, S=2048, D=1024, HPC=2, use_f32r=True, n_cores=8,
                      repeat=1, phases=(1, 2))
        _STATE["runner"] = Runner(nc, n_cores=8)
    return _STATE["runner"]


def kernel(x, Wq, bq, Wk, bk, Wv, bv, Wo, bo):
    import numpy as _np
    x = _np.asarray(x, dtype=_np.float32)
    Wq = _np.asarray(Wq, dtype=_np.float32)
    bq_ = _np.asarray(bq, dtype=_np.float32)
    Wk = _np.asarray(Wk, dtype=_np.float32)
    bk_ = _np.asarray(bk, dtype=_np.float32)
    Wv = _np.asarray(Wv, dtype=_np.float32)
    bv_ = _np.asarray(bv, dtype=_np.float32)
    Wo = _np.asarray(Wo, dtype=_np.float32)
    bo_ = _np.asarray(bo, dtype=_np.float32)
    B, S, D = x.shape
    r = _get_runner()
    maps = host_inputs(x, Wq, bq_, Wk, bk_, Wv, bv_, Wo)
    staged = r.stage_inputs(maps)
    outs = r.run(staged, r.make_zeros())
    res = r.results(outs)
    acc = _np.zeros((B * S, D), dtype=_np.float32)
    for c in range(8):
        acc += res[c]["po"].astype(_np.float32)
    return (acc.reshape(B, S, D) + bo_).astype(_np.float32)

